# revision 13
# baseline (speedup 1.0000x reference)
"""MoE layer (dense all-expert routing) Trainium2 Bass kernel.

Problem: x[4,2048,1024] f32, gate_w[1024,8], gate_b[8], expert_w[8,1024,1024].
  gate = softmax(x @ gate_w + gate_b)                  # [B,S,E]
  out  = einsum('bse,bseo->bso', gate, einsum('bsi,eio->bseo', x, expert_w))

Sharding: data-parallel over tokens. 8192 tokens split into 8 shards of 1024;
each core computes its shard against all 8 experts (weights replicated).
No collectives; host concatenates shard outputs.

Per-core kernel (v8):
  - all matmuls bf16 with f32 PSUM accumulation (rel err ~3e-3)
  - gate matmuls interleaved into expert 0's block so the in-order PE
    queue never blocks on the full x (old design stalled ~5us + HAM
    re-throttle). All 8 gate accumulation groups live in ONE PSUM bank
    as 8-col slices spaced 64 apart. PSUM start=True clears has_written
    for the WHOLE bank, so per-group starts would wipe each other; a
    single N=512 bias matmul (ones[1,128] x host-packed gb_rep[1,512],
    start=True) initializes data+has_written for the entire bank, and
    every gate matmul accumulates with start=False.
  - head DMA: x k-chunks on the two HWDGE rings, split so the columns
    phase A1 needs (tokens 0:640, m=0..4) land first; W0 k-chunks ride
    the (otherwise idle, ~190 GB/s) SWDGE queue ahead of the e1..e7
    weight stream.
  - e0 phase A1: k-outer over m=0..4 + gate rounds (m=0..4);
    phase A2: m=5..7 k-sweeps with that m's gate matmuls riding along.
  - combines: plain copies for m=0..2 at the head of the DVE queue (fast
    PSUM slot releases for m=7 and e1's first groups), then the gate is
    ready so every other init is one fused DVE op acc = psum * g0.
  - main loop e>=1: per m one 8-matmul PSUM group + one fused DVE op
    acc = (psum * g[:,e]) + acc; weights via SWDGE, 1MiB per (n,e).
  - tail: last two m-groups split into halves/quarters so the final
    combine + output DMA overlap the last matmuls.
"""

import numpy as np
import ml_dtypes
from contextlib import ExitStack

import concourse.bacc as bacc
import concourse.bass as bass
import concourse.mybir as mybir
import concourse.tile as tile

BF16 = mybir.dt.bfloat16
F32 = mybir.dt.float32

P = 128  # partitions
GSP = 64  # gate group column spacing inside the gate PSUM bank


def build_moe_nc(T=1024, D=1024, O=1024, E=8, NO=512, w_bufs=3, acc_bufs=16,
                 warmup_mms=6, a1_m=5):
    """Build the per-core Bass program.

    T: tokens per core, D: d_in, O: d_out, E: experts, NO: d_out tile (<=512).
    a1_m: number of m-groups in expert-0 phase A1 (k-outer, DMA-paced).
    """
    KT = D // P   # k tiles (contraction)
    MT = T // P   # token tiles
    NT = O // NO  # d_out tiles

    nc = bacc.Bacc("TRN2", target_bir_lowering=False, debug=False)
    xT_d = nc.dram_tensor("xT", [D, T], BF16, kind="ExternalInput")
    w_d = nc.dram_tensor("w", [E, D, O], BF16, kind="ExternalInput")
    # gwt[p, k*E+e] = gate_w[k*128+p, e]  (host pre-tiled, contiguous DMA)
    gwt_d = nc.dram_tensor("gwt", [P, KT * E], BF16, kind="ExternalInput")
    # gb_rep[0, m*GSP+e] = gate_b[e]  (bias row for all 8 gate groups)
    gbrep_d = nc.dram_tensor("gbrep", [1, NO], BF16, kind="ExternalInput")
    out_d = nc.dram_tensor("out", [T, O], F32, kind="ExternalOutput")

    ACUT = a1_m * P  # token column where the A1/A2 x-chunk split lies

    with tile.TileContext(nc) as tc:
        with ExitStack() as ctx:
            singles = ctx.enter_context(tc.tile_pool(name="singles", bufs=1))
            wpool = ctx.enter_context(tc.tile_pool(name="w", bufs=w_bufs))
            accp = ctx.enter_context(tc.tile_pool(name="acc", bufs=acc_bufs))
            gpool = ctx.enter_context(tc.tile_pool(name="gate", bufs=1))
            ps = ctx.enter_context(tc.tile_pool(name="ps", bufs=7, space="PSUM"))

            ones_t = singles.tile([1, P], BF16, tag="ones")
            nc.vector.memset(ones_t, 1.0)

            # gate consts first on the scalar ring — tiny, and the bias
            # matmul right after warmup depends on gb_rep
            gbrep_sb = singles.tile([1, NO], BF16, tag="gbrep")
            nc.scalar.dma_start(out=gbrep_sb, in_=gbrep_d[:, :])
            gw_t = singles.tile([P, KT, E], BF16, tag="gw")
            nc.scalar.dma_start(
                out=gw_t, in_=gwt_d[:, :].rearrange("p (k e) -> p k e", e=E)
            )

            # HAM warm-up: dummy matmuls on a memset tile keep the PE busy
            # while the first input DMAs are in flight. Results never read.
            warm = singles.tile([P, NO], BF16, tag="warm")
            nc.vector.memset(warm, 0.0)
            psw = ps.tile([P, NO], F32, tag="psg", bufs=1, name="psw")
            for j in range(warmup_mms):
                nc.tensor.matmul(
                    psw, lhsT=warm[:, 0:P], rhs=warm,
                    start=(j == 0), stop=(j == warmup_mms - 1),
                )

            # Head loads. x k-chunks alternate the two HWDGE rings (sync:
            # even k, scalar: odd k), each split into the A1 column range
            # (tokens 0:ACUT — needed first, k-paced) and the rest (needed
            # only from phase A2). W0's k-chunks go on the SWDGE queue,
            # ahead of the e1.. weight stream.
            wt0 = wpool.tile([P, KT, NO], BF16, tag="w", name="wt0")
            w0_src = w_d[0, :, 0:NO].rearrange("(k p) o -> p k o", p=P)
            xsrc = xT_d.rearrange("(k p) t -> p k t", p=P)
            xc = []
            for k in range(KT):
                t = singles.tile([P, 1, T], BF16, tag=f"xT{k}", name=f"xc{k}")
                xc.append(t)
            for k in range(KT):  # A1 column ranges, in k order
                eng = nc.sync if k % 2 == 0 else nc.scalar
                eng.dma_start(
                    out=xc[k][:, :, 0:ACUT], in_=xsrc[:, k:k + 1, 0:ACUT]
                )
                nc.gpsimd.dma_start(
                    out=wt0[:, k:k + 1, :], in_=w0_src[:, k:k + 1, :]
                )
            for k in range(KT):  # A2 column ranges
                eng = nc.sync if k % 2 == 0 else nc.scalar
                eng.dma_start(
                    out=xc[k][:, :, ACUT:T], in_=xsrc[:, k:k + 1, ACUT:T]
                )

            def xT(k):
                return xc[k][:, 0, :]

            # ---- gate state ------------------------------------------------
            # One PSUM bank holds all 8 gate accumulation groups as 8-wide
            # column slices spaced GSP apart. The single bias matmul below
            # (start=True over the full bank) initializes data+has_written.
            psg = ps.tile([P, NO], F32, tag="psg", bufs=1, name="psg")
            nc.tensor.matmul(
                psg, lhsT=ones_t, rhs=gbrep_sb, start=True, stop=False,
                skip_group_check=True,
            )

            def gslice(m):
                return psg[:, m * GSP:m * GSP + E]

            def gate_mm(m, k):
                nc.tensor.matmul(
                    gslice(m),
                    lhsT=xT(k)[:, m * P:(m + 1) * P],
                    rhs=gw_t[:, k, :],
                    start=False,
                    stop=(k == KT - 1),
                    skip_group_check=True,
                )

            g_sb = [None] * MT

            def emit_gate_dve(m):
                p_t = gpool.tile([P, E], F32, tag=f"p{m}", name=f"p{m}")
                s_t = gpool.tile([P, 1], F32, tag=f"s{m}", name=f"s{m}")
                # exp(logits); |logits| <~ 3 so no max-subtraction needed
                nc.scalar.activation(
                    p_t, gslice(m), mybir.ActivationFunctionType.Exp,
                    accum_out=s_t,
                )
                rs_t = gpool.tile([P, 1], F32, tag=f"rs{m}", name=f"rs{m}")
                nc.vector.reciprocal(rs_t, s_t)
                g_t = gpool.tile([P, E], F32, tag=f"g{m}", name=f"g{m}")
                nc.vector.tensor_scalar_mul(g_t, p_t, rs_t)
                g_sb[m] = g_t

            # ---- main: all-expert GEMM + fused gate combine ---------------
            for n in range(NT):
                acc = [None] * MT
                for e in range(E):
                    if n == 0 and e == 0:
                        wt = wt0
                    else:
                        wt = wpool.tile([P, KT, NO], BF16, tag="w")
                        nc.gpsimd.dma_start(
                            out=wt,
                            in_=w_d[e, :, n * NO:(n + 1) * NO].rearrange(
                                "(k p) o -> p k o", p=P
                            ),
                        )
                    if n == 0 and e == 0:
                        # ---- expert 0, phase A1: k-outer over m=0..a1_m-1,
                        # paced by the arriving x/W0 k-chunks; gate rounds
                        # for the same m ride along.
                        psy_l = [None] * MT
                        psy_l[a1_m] = ps.tile(
                            [P, NO], F32, tag="ps", name=f"psk{a1_m}"
                        )
                        for k in range(KT):
                            for m in range(a1_m):
                                if k == 0:
                                    psy_l[m] = ps.tile(
                                        [P, NO], F32, tag="ps", name=f"psk{m}"
                                    )
                                nc.tensor.matmul(
                                    psy_l[m],
                                    lhsT=xT(k)[:, m * P:(m + 1) * P],
                                    rhs=wt[:, k, :],
                                    start=(k == 0),
                                    stop=(k == KT - 1),
                                )
                            for m in range(a1_m):
                                gate_mm(m, k)
                            if k < KT - 1:
                                # HAM/jitter fillers: junk matmuls into the
                                # phase-A2 slot (reset by its start=True)
                                for _ in range(2):
                                    nc.tensor.matmul(
                                        psy_l[a1_m][:, 0:P],
                                        lhsT=warm[:, 0:P],
                                        rhs=warm[:, 0:P],
                                        start=False,
                                        stop=False,
                                        skip_group_check=True,
                                    )
                        # m=0..2: plain copies first — fast PSUM slot
                        # releases at the head of the DVE queue (m=7's
                        # sweep and e1's first groups reuse these slots);
                        # scale by g0 later once the gate is ready
                        for m in range(3):
                            acc[m] = accp.tile(
                                [P, NO], F32, tag="acc", name=f"acc{m}"
                            )
                            nc.vector.tensor_copy(acc[m], psy_l[m])
                        for m in range(a1_m):
                            emit_gate_dve(m)
                        # m=3..a1_m-1: gate ready — single fused scale+init
                        for m in range(3, a1_m):
                            acc[m] = accp.tile(
                                [P, NO], F32, tag="acc", name=f"acc{m}"
                            )
                            nc.vector.tensor_scalar_mul(
                                acc[m], psy_l[m], g_sb[m][:, 0:1]
                            )
                        for m in range(3):
                            nc.vector.tensor_scalar_mul(
                                acc[m], acc[m], g_sb[m][:, 0:1]
                            )
                        # ---- phase A2: m=a1_m..7 k-sweeps, gate rides along
                        for m in range(a1_m, MT):
                            if m == MT - 1:
                                # dummy generation: advances the pool cursor
                                # so psk7 reuses the slot copy(acc0) freed
                                ps.tile([P, NO], F32, tag="ps", name="ps_dum")
                            if psy_l[m] is None:
                                psy_l[m] = ps.tile(
                                    [P, NO], F32, tag="ps", name=f"psk{m}"
                                )
                            for k in range(KT):
                                nc.tensor.matmul(
                                    psy_l[m],
                                    lhsT=xT(k)[:, m * P:(m + 1) * P],
                                    rhs=wt[:, k, :],
                                    start=(k == 0),
                                    stop=(k == KT - 1),
                                )
                            # gate matmuls as one consecutive block — they
                            # pipeline at ~55ns each vs ~130ns when each
                            # trails a 512-wide matmul
                            for k in range(KT):
                                gate_mm(m, k)
                            emit_gate_dve(m)
                            acc[m] = accp.tile(
                                [P, NO], F32, tag="acc", name=f"acc{m}"
                            )
                            nc.vector.tensor_scalar_mul(
                                acc[m], psy_l[m], g_sb[m][:, 0:1]
                            )
                        continue
                    for m in range(MT):
                        if n == NT - 1 and e == E - 1 and m >= MT - 2:
                            # tail: split the last two m-groups (halves then
                            # quarters) so combine + output DMA overlap the
                            # final matmuls
                            nh = 2 if m == MT - 2 else 4
                            NH = NO // nh
                            acch = acc[m]
                            for h in range(nh):
                                psy_h = ps.tile(
                                    [P, NH], F32, tag="ps", name=f"psyh{m}_{h}"
                                )
                                for k in range(KT):
                                    nc.tensor.matmul(
                                        psy_h,
                                        lhsT=xT(k)[:, m * P:(m + 1) * P],
                                        rhs=wt[:, k, h * NH:(h + 1) * NH],
                                        start=(k == 0),
                                        stop=(k == KT - 1),
                                    )
                                nc.vector.scalar_tensor_tensor(
                                    out=acch[:, h * NH:(h + 1) * NH],
                                    in0=psy_h,
                                    scalar=g_sb[m][:, e:e + 1],
                                    in1=acch[:, h * NH:(h + 1) * NH],
                                    op0=mybir.AluOpType.mult,
                                    op1=mybir.AluOpType.add,
                                )
                                # split each piece across both rings so the
                                # final drains never serialize on one queue
                                HH = NH // 2
                                for q, eng in ((0, nc.sync), (1, nc.scalar)):
                                    lo = h * NH + q * HH
                                    eng.dma_start(
                                        out=out_d[
                                            m * P:(m + 1) * P,
                                            n * NO + lo:n * NO + lo + HH,
                                        ],
                                        in_=acch[:, lo:lo + HH],
                                    )
                            continue
                        psy = ps.tile([P, NO], F32, tag="ps", name=f"psy{m}")
                        for k in range(KT):
                            nc.tensor.matmul(
                                psy,
                                lhsT=xT(k)[:, m * P:(m + 1) * P],
                                rhs=wt[:, k, :],
                                start=(k == 0),
                                stop=(k == KT - 1),
                            )
                        if e == 0:
                            # n=1 expert 0: gate ready — fused scale+init
                            acc[m] = accp.tile(
                                [P, NO], F32, tag="acc", name=f"acc{m}"
                            )
                            nc.vector.tensor_scalar_mul(
                                acc[m], psy, g_sb[m][:, 0:1]
                            )
                        else:
                            nc.vector.scalar_tensor_tensor(
                                out=acc[m],
                                in0=psy,
                                scalar=g_sb[m][:, e:e + 1],
                                in1=acc[m],
                                op0=mybir.AluOpType.mult,
                                op1=mybir.AluOpType.add,
                            )
                        if e == E - 1:
                            if n == NT - 1:
                                # last block: halve across both rings so
                                # neither queue backs up into the tail
                                for q, eng in ((0, nc.sync), (1, nc.scalar)):
                                    lo = q * (NO // 2)
                                    eng.dma_start(
                                        out=out_d[
                                            m * P:(m + 1) * P,
                                            n * NO + lo:n * NO + lo + NO // 2,
                                        ],
                                        in_=acc[m][:, lo:lo + NO // 2],
                                    )
                            else:
                                eng = nc.sync if m % 2 == 0 else nc.scalar
                                eng.dma_start(
                                    out=out_d[
                                        m * P:(m + 1) * P, n * NO:(n + 1) * NO
                                    ],
                                    in_=acc[m],
                                )
    nc.compile()
    return nc


# ---------------------------------------------------------------------------
# Host wrapper: full inputs -> shard -> run SPMD on 8 cores -> gather
# ---------------------------------------------------------------------------

N_CORES = 8
_B, _S, _DIN, _DOUT, _E = 4, 2048, 1024, 1024, 8


def _host_gwt(gate_w):
    """[D, E] -> [128, KT*E] with gwt[p, k*E+e] = gate_w[k*128+p, e]."""
    D, E = gate_w.shape
    kt = D // P
    return np.ascontiguousarray(
        gate_w.reshape(kt, P, E).transpose(1, 0, 2).reshape(P, kt * E)
    )


def _host_gbrep(gate_b, NO=512):
    """[E] -> [1, NO] with gb_rep[0, m*GSP+e] = gate_b[e], 0 elsewhere."""
    E = gate_b.shape[0]
    rep = np.zeros((1, NO), dtype=np.float32)
    for m in range(NO // GSP):
        rep[0, m * GSP:m * GSP + E] = gate_b
    return rep


LAST_RESULTS = None  # BassKernelResults of the most recent run (for profiling)


def kernel(x, gate_w, gate_b, expert_w, _trace=False):
    global LAST_RESULTS
    from concourse.bass_utils import run_bass_kernel_spmd

    x = np.asarray(x)
    tokens = x.reshape(-1, _DIN)  # [8192, 1024]
    n_tok = tokens.shape[0]
    tpc = n_tok // N_CORES  # tokens per core

    w_bf = np.asarray(expert_w, dtype=ml_dtypes.bfloat16)
    gwt_bf = _host_gwt(np.asarray(gate_w)).astype(ml_dtypes.bfloat16)
    gbrep_bf = _host_gbrep(np.asarray(gate_b)).astype(ml_dtypes.bfloat16)

    in_maps = []
    for c in range(N_CORES):
        shard = tokens[c * tpc:(c + 1) * tpc]  # [1024, 1024]
        xT = np.ascontiguousarray(shard.T).astype(ml_dtypes.bfloat16)
        in_maps.append({"xT": xT, "w": w_bf, "gwt": gwt_bf, "gbrep": gbrep_bf})

    nc = build_moe_nc(T=tpc, D=_DIN, O=_DOUT, E=_E)
    res = run_bass_kernel_spmd(nc, in_maps, list(range(N_CORES)), trace=_trace)
    LAST_RESULTS = res
    outs = [res.results[c]["out"] for c in range(N_CORES)]
    full = np.concatenate(outs, axis=0).astype(np.float32)
    return full.reshape(_B, _S, _DOUT)


# revision 16
# speedup vs baseline: 1.0115x; 1.0115x over previous
"""MoE layer (dense all-expert routing) Trainium2 Bass kernel.

Problem: x[4,2048,1024] f32, gate_w[1024,8], gate_b[8], expert_w[8,1024,1024].
  gate = softmax(x @ gate_w + gate_b)                  # [B,S,E]
  out  = einsum('bse,bseo->bso', gate, einsum('bsi,eio->bseo', x, expert_w))

Sharding: data-parallel over tokens. 8192 tokens split into 8 shards of 1024;
each core computes its shard against all 8 experts (weights replicated).
No collectives; host concatenates shard outputs.

Per-core kernel (v9):
  - all matmuls bf16 with f32 PSUM accumulation (rel err ~3e-3)
  - gate matmuls interleaved into expert 0's block so the in-order PE
    queue never blocks on the full x (old design stalled ~5us + HAM
    re-throttle). All 8 gate accumulation groups live in ONE PSUM bank
    as 8-col slices spaced 64 apart. PSUM start=True clears has_written
    for the WHOLE bank, so per-group starts would wipe each other; a
    single N=512 bias matmul (ones[1,128] x host-packed gb_rep[1,512],
    start=True) initializes data+has_written for the entire bank, and
    every gate matmul accumulates with start=False.
  - head DMA: x k-chunks on the two HWDGE rings, split so the columns
    phase A1 needs (tokens 0:640, m=0..4) land first; W0 k-chunks ride
    the (otherwise idle, ~190 GB/s) SWDGE queue ahead of the e1..e7
    weight stream.
  - e0 phase A1: k-outer over m=0..4 + gate rounds (m=0..4);
    phase A2: m=5..7 k-sweeps with that m's gate matmuls riding along.
  - combines: plain copies for m=0..2 at the head of the DVE queue (fast
    PSUM slot releases for m=7 and e1's first groups), then the gate is
    ready so every other init is one fused DVE op acc = psum * g0.
  - main loop e>=1: per m one 8-matmul PSUM group + one fused DVE op
    acc = (psum * g[:,e]) + acc; weights via SWDGE, 1MiB per (n,e).
  - tail: last two m-groups split into halves/quarters so the final
    combine + output DMA overlap the last matmuls.
"""

import numpy as np
import ml_dtypes
from contextlib import ExitStack

import concourse.bacc as bacc
import concourse.bass as bass
import concourse.mybir as mybir
import concourse.tile as tile

BF16 = mybir.dt.bfloat16
F32 = mybir.dt.float32

P = 128  # partitions
GSP = 64  # gate group column spacing inside the gate PSUM bank


def build_moe_nc(T=1024, D=1024, O=1024, E=8, NO=512, w_bufs=3, acc_bufs=16,
                 warmup_mms=6, a1_m=5):
    """Build the per-core Bass program.

    T: tokens per core, D: d_in, O: d_out, E: experts, NO: d_out tile (<=512).
    a1_m: number of m-groups in expert-0 phase A1 (k-outer, DMA-paced).
    """
    KT = D // P   # k tiles (contraction)
    MT = T // P   # token tiles
    NT = O // NO  # d_out tiles

    nc = bacc.Bacc("TRN2", target_bir_lowering=False, debug=False)
    xT_d = nc.dram_tensor("xT", [D, T], BF16, kind="ExternalInput")
    w_d = nc.dram_tensor("w", [E, D, O], BF16, kind="ExternalInput")
    # gwt[p, k*E+e] = gate_w[k*128+p, e]  (host pre-tiled, contiguous DMA)
    gwt_d = nc.dram_tensor("gwt", [P, KT * E], BF16, kind="ExternalInput")
    # gb_rep[0, m*GSP+e] = gate_b[e]  (bias row for all 8 gate groups)
    gbrep_d = nc.dram_tensor("gbrep", [1, NO], BF16, kind="ExternalInput")
    out_d = nc.dram_tensor("out", [T, O], F32, kind="ExternalOutput")

    ACUT = a1_m * P  # token column where the A1/A2 x-chunk split lies

    with tile.TileContext(nc) as tc:
        with ExitStack() as ctx:
            singles = ctx.enter_context(tc.tile_pool(name="singles", bufs=1))
            wpool = ctx.enter_context(tc.tile_pool(name="w", bufs=w_bufs))
            accp = ctx.enter_context(tc.tile_pool(name="acc", bufs=acc_bufs))
            gpool = ctx.enter_context(tc.tile_pool(name="gate", bufs=1))
            ps = ctx.enter_context(tc.tile_pool(name="ps", bufs=7, space="PSUM"))

            ones_t = singles.tile([1, P], BF16, tag="ones")
            nc.vector.memset(ones_t, 1.0)

            # gate consts first on the scalar ring — tiny, and the bias
            # matmul right after warmup depends on gb_rep
            gbrep_sb = singles.tile([1, NO], BF16, tag="gbrep")
            nc.scalar.dma_start(out=gbrep_sb, in_=gbrep_d[:, :])
            gw_t = singles.tile([P, KT, E], BF16, tag="gw")
            nc.scalar.dma_start(
                out=gw_t, in_=gwt_d[:, :].rearrange("p (k e) -> p k e", e=E)
            )

            # HAM warm-up: dummy matmuls on a memset tile keep the PE busy
            # while the first input DMAs are in flight. Results never read.
            warm = singles.tile([P, NO], BF16, tag="warm")
            nc.vector.memset(warm, 0.0)
            psw = ps.tile([P, NO], F32, tag="psg", bufs=1, name="psw")
            for j in range(warmup_mms):
                nc.tensor.matmul(
                    psw, lhsT=warm[:, 0:P], rhs=warm,
                    start=(j == 0), stop=(j == warmup_mms - 1),
                )

            # Head loads. x k-chunks alternate the two HWDGE rings (sync:
            # even k, scalar: odd k), each split into the A1 column range
            # (tokens 0:ACUT — needed first, k-paced) and the rest (needed
            # only from phase A2). W0's k-chunks go on the SWDGE queue,
            # ahead of the e1.. weight stream.
            wt0 = wpool.tile([P, KT, NO], BF16, tag="w", name="wt0")
            w0_src = w_d[0, :, 0:NO].rearrange("(k p) o -> p k o", p=P)
            xsrc = xT_d.rearrange("(k p) t -> p k t", p=P)
            xc = []
            for k in range(KT):
                t = singles.tile([P, 1, T], BF16, tag=f"xT{k}", name=f"xc{k}")
                xc.append(t)
            for k in range(KT):  # A1 column ranges, in k order
                eng = nc.sync if k % 2 == 0 else nc.scalar
                eng.dma_start(
                    out=xc[k][:, :, 0:ACUT], in_=xsrc[:, k:k + 1, 0:ACUT]
                )
                nc.gpsimd.dma_start(
                    out=wt0[:, k:k + 1, :], in_=w0_src[:, k:k + 1, :]
                )
            for k in range(KT):  # A2 column ranges
                eng = nc.sync if k % 2 == 0 else nc.scalar
                eng.dma_start(
                    out=xc[k][:, :, ACUT:T], in_=xsrc[:, k:k + 1, ACUT:T]
                )

            def xT(k):
                return xc[k][:, 0, :]

            # ---- gate state ------------------------------------------------
            # One PSUM bank holds all 8 gate accumulation groups as 8-wide
            # column slices spaced GSP apart. The single bias matmul below
            # (start=True over the full bank) initializes data+has_written.
            psg = ps.tile([P, NO], F32, tag="psg", bufs=1, name="psg")
            nc.tensor.matmul(
                psg, lhsT=ones_t, rhs=gbrep_sb, start=True, stop=False,
                skip_group_check=True,
            )

            def gslice(m):
                return psg[:, m * GSP:m * GSP + E]

            def gate_mm(m, k):
                nc.tensor.matmul(
                    gslice(m),
                    lhsT=xT(k)[:, m * P:(m + 1) * P],
                    rhs=gw_t[:, k, :],
                    start=False,
                    stop=(k == KT - 1),
                    skip_group_check=True,
                )

            g_sb = [None] * MT

            def emit_gate_dve(m):
                p_t = gpool.tile([P, E], F32, tag=f"p{m}", name=f"p{m}")
                s_t = gpool.tile([P, 1], F32, tag=f"s{m}", name=f"s{m}")
                # exp(logits); |logits| <~ 3 so no max-subtraction needed
                nc.scalar.activation(
                    p_t, gslice(m), mybir.ActivationFunctionType.Exp,
                    accum_out=s_t,
                )
                rs_t = gpool.tile([P, 1], F32, tag=f"rs{m}", name=f"rs{m}")
                nc.vector.reciprocal(rs_t, s_t)
                g_t = gpool.tile([P, E], F32, tag=f"g{m}", name=f"g{m}")
                nc.vector.tensor_scalar_mul(g_t, p_t, rs_t)
                g_sb[m] = g_t

            # ---- main: all-expert GEMM + fused gate combine ---------------
            for n in range(NT):
                acc = [None] * MT
                for e in range(E):
                    if n == 0 and e == 0:
                        wt = wt0
                    else:
                        wt = wpool.tile([P, KT, NO], BF16, tag="w")
                        nc.gpsimd.dma_start(
                            out=wt,
                            in_=w_d[e, :, n * NO:(n + 1) * NO].rearrange(
                                "(k p) o -> p k o", p=P
                            ),
                        )
                    if n == 0 and e == 0:
                        # ---- expert 0, phase A1: k-outer over m=0..a1_m-1,
                        # paced by the arriving x/W0 k-chunks; gate rounds
                        # for the same m ride along.
                        psy_l = [None] * MT
                        psy_l[a1_m] = ps.tile(
                            [P, NO], F32, tag="ps", name=f"psk{a1_m}"
                        )
                        for k in range(KT):
                            for m in range(a1_m):
                                if k == 0:
                                    psy_l[m] = ps.tile(
                                        [P, NO], F32, tag="ps", name=f"psk{m}"
                                    )
                                nc.tensor.matmul(
                                    psy_l[m],
                                    lhsT=xT(k)[:, m * P:(m + 1) * P],
                                    rhs=wt[:, k, :],
                                    start=(k == 0),
                                    stop=(k == KT - 1),
                                )
                            for m in range(a1_m):
                                gate_mm(m, k)
                            if k < KT - 1:
                                # HAM/jitter fillers: junk matmuls into the
                                # phase-A2 slot (reset by its start=True)
                                for _ in range(2):
                                    nc.tensor.matmul(
                                        psy_l[a1_m][:, 0:P],
                                        lhsT=warm[:, 0:P],
                                        rhs=warm[:, 0:P],
                                        start=False,
                                        stop=False,
                                        skip_group_check=True,
                                    )
                        # m=0..2: plain copies first — fast PSUM slot
                        # releases at the head of the DVE queue (m=7's
                        # sweep and e1's first groups reuse these slots);
                        # scale by g0 later once the gate is ready
                        for m in range(3):
                            acc[m] = accp.tile(
                                [P, NO], F32, tag="acc", name=f"acc{m}"
                            )
                            nc.vector.tensor_copy(acc[m], psy_l[m])
                        for m in range(a1_m):
                            emit_gate_dve(m)
                        # m=3..a1_m-1: gate ready — single fused scale+init
                        for m in range(3, a1_m):
                            acc[m] = accp.tile(
                                [P, NO], F32, tag="acc", name=f"acc{m}"
                            )
                            nc.vector.tensor_scalar_mul(
                                acc[m], psy_l[m], g_sb[m][:, 0:1]
                            )
                        for m in range(3):
                            nc.vector.tensor_scalar_mul(
                                acc[m], acc[m], g_sb[m][:, 0:1]
                            )
                        # ---- phase A2: m=a1_m..7 k-sweeps, gate rides along
                        for m in range(a1_m, MT):
                            if m == MT - 1:
                                # dummy generation: advances the pool cursor
                                # so psk7 reuses the slot copy(acc0) freed
                                ps.tile([P, NO], F32, tag="ps", name="ps_dum")
                            if psy_l[m] is None:
                                psy_l[m] = ps.tile(
                                    [P, NO], F32, tag="ps", name=f"psk{m}"
                                )
                            for k in range(KT):
                                nc.tensor.matmul(
                                    psy_l[m],
                                    lhsT=xT(k)[:, m * P:(m + 1) * P],
                                    rhs=wt[:, k, :],
                                    start=(k == 0),
                                    stop=(k == KT - 1),
                                )
                            # gate matmuls as one consecutive block — they
                            # pipeline at ~55ns each vs ~130ns when each
                            # trails a 512-wide matmul
                            for k in range(KT):
                                gate_mm(m, k)
                            emit_gate_dve(m)
                            acc[m] = accp.tile(
                                [P, NO], F32, tag="acc", name=f"acc{m}"
                            )
                            nc.vector.tensor_scalar_mul(
                                acc[m], psy_l[m], g_sb[m][:, 0:1]
                            )
                        continue
                    for m in range(MT):
                        if n == NT - 1 and e == E - 1 and m >= MT - 2:
                            # tail: split the last two m-groups (halves then
                            # quarters) so combine + output DMA overlap the
                            # final matmuls
                            nh = 2 if m == MT - 2 else 4
                            NH = NO // nh
                            acch = acc[m]
                            for h in range(nh):
                                psy_h = ps.tile(
                                    [P, NH], F32, tag="ps", name=f"psyh{m}_{h}"
                                )
                                for k in range(KT):
                                    nc.tensor.matmul(
                                        psy_h,
                                        lhsT=xT(k)[:, m * P:(m + 1) * P],
                                        rhs=wt[:, k, h * NH:(h + 1) * NH],
                                        start=(k == 0),
                                        stop=(k == KT - 1),
                                    )
                                nc.vector.scalar_tensor_tensor(
                                    out=acch[:, h * NH:(h + 1) * NH],
                                    in0=psy_h,
                                    scalar=g_sb[m][:, e:e + 1],
                                    in1=acch[:, h * NH:(h + 1) * NH],
                                    op0=mybir.AluOpType.mult,
                                    op1=mybir.AluOpType.add,
                                )
                                eng = nc.sync if h % 2 == 0 else nc.scalar
                                eng.dma_start(
                                    out=out_d[
                                        m * P:(m + 1) * P,
                                        n * NO + h * NH:n * NO + (h + 1) * NH,
                                    ],
                                    in_=acch[:, h * NH:(h + 1) * NH],
                                )
                            continue
                        psy = ps.tile([P, NO], F32, tag="ps", name=f"psy{m}")
                        for k in range(KT):
                            nc.tensor.matmul(
                                psy,
                                lhsT=xT(k)[:, m * P:(m + 1) * P],
                                rhs=wt[:, k, :],
                                start=(k == 0),
                                stop=(k == KT - 1),
                            )
                        if e == 0:
                            # n=1 expert 0: gate ready — fused scale+init
                            acc[m] = accp.tile(
                                [P, NO], F32, tag="acc", name=f"acc{m}"
                            )
                            nc.vector.tensor_scalar_mul(
                                acc[m], psy, g_sb[m][:, 0:1]
                            )
                        else:
                            nc.vector.scalar_tensor_tensor(
                                out=acc[m],
                                in0=psy,
                                scalar=g_sb[m][:, e:e + 1],
                                in1=acc[m],
                                op0=mybir.AluOpType.mult,
                                op1=mybir.AluOpType.add,
                            )
                        if e == E - 1:
                            eng = nc.sync if m % 2 == 0 else nc.scalar
                            eng.dma_start(
                                out=out_d[
                                    m * P:(m + 1) * P, n * NO:(n + 1) * NO
                                ],
                                in_=acc[m],
                            )
    nc.compile()
    return nc


# ---------------------------------------------------------------------------
# Host wrapper: full inputs -> shard -> run SPMD on 8 cores -> gather
# ---------------------------------------------------------------------------

N_CORES = 8
_B, _S, _DIN, _DOUT, _E = 4, 2048, 1024, 1024, 8


def _host_gwt(gate_w):
    """[D, E] -> [128, KT*E] with gwt[p, k*E+e] = gate_w[k*128+p, e]."""
    D, E = gate_w.shape
    kt = D // P
    return np.ascontiguousarray(
        gate_w.reshape(kt, P, E).transpose(1, 0, 2).reshape(P, kt * E)
    )


def _host_gbrep(gate_b, NO=512):
    """[E] -> [1, NO] with gb_rep[0, m*GSP+e] = gate_b[e], 0 elsewhere."""
    E = gate_b.shape[0]
    rep = np.zeros((1, NO), dtype=np.float32)
    for m in range(NO // GSP):
        rep[0, m * GSP:m * GSP + E] = gate_b
    return rep


LAST_RESULTS = None  # BassKernelResults of the most recent run (for profiling)


def kernel(x, gate_w, gate_b, expert_w, _trace=False):
    global LAST_RESULTS
    from concourse.bass_utils import run_bass_kernel_spmd

    x = np.asarray(x)
    tokens = x.reshape(-1, _DIN)  # [8192, 1024]
    n_tok = tokens.shape[0]
    tpc = n_tok // N_CORES  # tokens per core

    w_bf = np.asarray(expert_w, dtype=ml_dtypes.bfloat16)
    gwt_bf = _host_gwt(np.asarray(gate_w)).astype(ml_dtypes.bfloat16)
    gbrep_bf = _host_gbrep(np.asarray(gate_b)).astype(ml_dtypes.bfloat16)

    in_maps = []
    for c in range(N_CORES):
        shard = tokens[c * tpc:(c + 1) * tpc]  # [1024, 1024]
        xT = np.ascontiguousarray(shard.T).astype(ml_dtypes.bfloat16)
        in_maps.append({"xT": xT, "w": w_bf, "gwt": gwt_bf, "gbrep": gbrep_bf})

    nc = build_moe_nc(T=tpc, D=_DIN, O=_DOUT, E=_E)
    res = run_bass_kernel_spmd(nc, in_maps, list(range(N_CORES)), trace=_trace)
    LAST_RESULTS = res
    outs = [res.results[c]["out"] for c in range(N_CORES)]
    full = np.concatenate(outs, axis=0).astype(np.float32)
    return full.reshape(_B, _S, _DOUT)
